# revision 1
# baseline (speedup 1.0000x reference)
"""Causal attention with ALiBi for Trainium2, tensor-parallel over heads x
data-parallel over batch (8 NeuronCores).

Problem: B=4, S=2048, D=2048, NH=16, HD=128, fp32.
  q/k/v = x @ Wq/Wk/Wv ; scores = q k^T / sqrt(HD) + alibi ; causal softmax ;
  out = (probs @ v) @ Wo

Sharding: core (b, j) handles batch b and the 8 interleaved heads
  j, j+2, ..., j+14.  Each core returns out_partial^T; the host sums the two
  per-batch partials and transposes back.

Implementation notes (v2):
  * x^T is built on the HOST and shipped as hi/lo fp8e4 pairs (x = hi + lo,
    lo = fp8(x - hi)); Wq/Wk/Wv likewise (scaled by 128 to stay in the fp8
    normal range).  QKV projections run as fp8 DoubleRow matmuls (2 k-planes
    per instruction, 256-deep contraction at 157 TF/s):
      q = x_hi@W_hi  (8 instrs, chunk pairs)
        + x_lo@W_hi + x_hi@W_lo  (16 instrs, one per 128-chunk, the two
          cross terms paired as the two planes of one DoubleRow instr)
    The lo*lo term (~2^-8 relative) is dropped; projection accuracy ~bf16.
  * V is produced directly in [s, hd] layout (XT as the stationary operand),
    so there are no PE transposes anywhere.
  * Attention interior is bf16: scores = K^T Q per [128k x 512q] block,
    exp on ACT (bias column carries the ALiBi + per-q-tile shift constant;
    the two steepest local heads apply an exact per-q shift on DVE first),
    P@V accumulated in PSUM.
  * Softmax sums use e_sb as the matmul STATIONARY with a ones column as
    the moving operand: out [128q-chunk, 1] per chunk, accumulated in PSUM
    across k-blocks.  The ones column carries the value 128 so the V scale
    (weights pre-scaled by 128) cancels in pot/sums.
    The per-(h,qt) reciprocal row is broadcast across partitions via a DRAM
    bounce (SBUF[128,4] -> DRAM[512] -> partition_broadcast DMA).
  * O stays in SBUF (bf16, all heads); out^T = Wo^T @ O^T in bf16.
  * ALiBi block skipping as in v1 (blocks with decay < ~e^-18 of the softmax
    sum are skipped; fully-masked prefixes of diagonal blocks trimmed;
    partial bands masked by a DVE triangular-mask multiply, except the two
    steepest local heads where masked entries overflow exp and a gpsimd
    affine_select is used instead).
  * Scheduling: x^T arrives in four s-quarter DMAs so the first projection
    group starts ~7us in; weights are prefetched two heads ahead; each
    head's projection is EMITTED INTERLEAVED into the previous head's
    attention block loop (a generator-feeder), filling the ACT exp latency
    bubbles on the PE; the Wo stage is interleaved into the last head's
    attention per s-tile.  Measured (TimelineSim): ~451 us vs 735 us for
    the f32r v1 baseline, PE ~93% busy.
"""

import math

import numpy as np

B, S, D, NH = 4, 2048, 2048, 16
HD = D // NH            # 128
NHG = NH // 2           # heads per core
DC = D // 128           # 16 d-chunks
DC2 = DC // 2           # 8 256-wide d-chunks
QT_TILES = S // 512     # 4 q tiles
SCALE = 1.0 / math.sqrt(HD)
WS = 128.0              # weight pre-scale for fp8
PSUM_SCALE = SCALE / (WS * WS)   # q,k both carry a factor WS

_cache = {}


def _get_slopes(n):
    def pow2(n):
        start = 2 ** (-(2 ** (-(math.log2(n) - 3))))
        return [start * start**i for i in range(n)]

    if math.log2(n).is_integer():
        return pow2(n)
    c = 2 ** math.floor(math.log2(n))
    return pow2(c) + _get_slopes(2 * c)[0::2][: n - c]


def _build():
    import concourse.bacc as bacc
    import concourse.mybir as mybir
    import concourse.tile as tile
    from concourse.bass import ts

    f32 = mybir.dt.float32
    bf16 = mybir.dt.bfloat16
    f8 = mybir.dt.float8e4
    Exp = mybir.ActivationFunctionType.Exp
    DR = mybir.MatmulPerfMode.DoubleRow

    nc = bacc.Bacc()
    # xt8[p, ((dc2*2 + c)*2 + hl)*S + s]; d = dc2*256 + c*128 + p; hl=(lo,hi)
    xt_in = nc.declare_dram_parameter("xt8", [128, DC2 * 2 * 2 * S], f8,
                                      isOutput=False)
    # w8[p, (((h*DC2 + dc2)*2 + c)*2 + hl)*HD + hd]; hl=(hi,lo)
    wq_in = nc.declare_dram_parameter("wq8", [128, NHG * DC2 * 2 * 2 * HD], f8,
                                      isOutput=False)
    wk_in = nc.declare_dram_parameter("wk8", [128, NHG * DC2 * 2 * 2 * HD], f8,
                                      isOutput=False)
    wv_in = nc.declare_dram_parameter("wv8", [128, NHG * DC2 * 2 * 2 * HD], f8,
                                      isOutput=False)
    # wo8[p, (h*2 + hl)*D + f]; row m = h*HD + p; hl=(hi,lo); scaled by WS
    wo_in = nc.declare_dram_parameter("wo8", [128, NHG * 2 * D], f8,
                                      isOutput=False)
    # alibi_b[p, ((h*16+kc)*4+qt)] = -slope_h*(S-1-(kc*128+p)) + C[h,qt]
    alibi_b_in = nc.declare_dram_parameter(
        "alibi_b", [128, NHG * DC * QT_TILES], f32, isOutput=False)
    # alibi_q[h, q] = +slope_h * (S-1 - q)   (per-query shift)
    alibi_q_in = nc.declare_dram_parameter("alibi_q", [NHG, S], f32,
                                           isOutput=False)
    ones_col_in = nc.declare_dram_parameter("ones_col", [128, 1], bf16,
                                            isOutput=False)
    # tri[p, j] = 1 if j >= p else 0 — causal mask for the diagonal
    # 128-band of every score block (local coords are the same for all)
    tri_in = nc.declare_dram_parameter("tri", [128, 128], bf16,
                                       isOutput=False)
    outT = nc.declare_dram_parameter("outT", [D, S], f32, isOutput=True)

    recip_dram = nc.dram_tensor("recip_scratch", [NHG, QT_TILES, 512], f32)

    with tile.TileContext(nc) as tc:
        with (
            tc.tile_pool(name="consts", bufs=1) as pc,
            tc.tile_pool(name="psA", bufs=2, space="PSUM") as psA,
            tc.tile_pool(name="psST", bufs=3, space="PSUM") as psST,
            tc.tile_pool(name="psB", bufs=1, space="PSUM") as psB,
        ):
            alibi_sb = pc.tile([128, NHG * DC * QT_TILES], f32,
                               name="alibi_sb")
            ones_col = pc.tile([128, 1], bf16, name="ones_col_sb")
            tri_sb = pc.tile([128, 128], bf16, name="tri_sb")
            # O as hi/lo fp8 pairs for the DoubleRow Wo stage; hl=(lo,hi)
            o_all = pc.tile([128, NHG, 2, S], f8, name="o_all")

            def load_consts():
                nc.sync.dma_start(alibi_sb[:], alibi_b_in[:])
                nc.sync.dma_start(ones_col[:], ones_col_in[:])
                nc.sync.dma_start(tri_sb[:], tri_in[:])

            with (
                tc.tile_pool(name="xt", bufs=1) as pxt,
                tc.tile_pool(name="wp", bufs=2) as pw,
                tc.tile_pool(name="qkv2", bufs=2) as pq2,
                tc.tile_pool(name="qkv", bufs=2) as pq,
                tc.tile_pool(name="att", bufs=2) as pa,
                tc.tile_pool(name="epool", bufs=4) as pe_pool,
                tc.tile_pool(name="small", bufs=2) as psm,
                tc.tile_pool(name="wo", bufs=1) as pwo,
                tc.tile_pool(name="ost", bufs=4) as post,
            ):
                # XT8[p, dc2, c, hl, s-slice] per s-quarter (hl: 0=lo, 1=hi);
                # one tile per 512-wide s-quarter so the first projection
                # group (st=0) only waits for the first DMA, and the proj
                # pipeline keeps pace with the arrivals.  Emitted after the
                # first head's (small) weight DMA, see below.
                XTq = [None] * QT_TILES

                def load_xt(quarters):
                    xin = xt_in.rearrange("p (a c l s) -> p a c l s",
                                          a=DC2, c=2, l=2)
                    for sq in quarters:
                        xt_t = pxt.tile([128, DC2, 2, 2, 512], f8,
                                        tag=f"xt{sq}", name=f"XT{sq}")
                        nc.sync.dma_start(
                            xt_t[:], xin[:, :, :, :, ts(sq, 512)])
                        XTq[sq] = xt_t

                def load_w(h, after_wq=None):
                    w_sb = pw.tile([128, 3, DC2, 2, 2, HD], f8, tag="w",
                                   name="w_sb")
                    for wi, w_in in enumerate((wq_in, wk_in, wv_in)):
                        nc.sync.dma_start(
                            w_sb[:, wi],
                            w_in[:, ts(h, DC2 * 2 * 2 * HD)].rearrange(
                                "p (a c l f) -> p a c l f", a=DC2, c=2, l=2))
                        if wi == 0 and after_wq is not None:
                            after_wq()
                    return w_sb

                def proj_gen(h, w_sb, out):
                    """Generator projecting q, k (as [hd, S] bf16) and v
                    ([s-chunk, sc, hd] bf16) for local head h via hi/lo fp8
                    DoubleRow.  Yields every ~12 matmuls so the driver can
                    interleave the emission into the previous head's
                    attention (filling ACT-latency bubbles on the PE)."""
                    qt_sb = pq2.tile([128, S], bf16, tag="QT", name="qt_sb")
                    kt_sb = pq2.tile([128, S], bf16, tag="KT", name="kt_sb")
                    v_sb = pq.tile([128, DC, HD], bf16, tag="V", name="v_sb")
                    out["q"], out["k"], out["v"] = qt_sb, kt_sb, v_sb

                    for st in range(QT_TILES):
                        parts = [(XTq[st], 0, 512)]
                        # q^T, k^T: stationary = W chunk, moving = XT
                        for wi, dst in ((0, qt_sb), (1, kt_sb)):
                            for xt_t, col0, w_cols in parts:
                                pp = psA.tile([128, w_cols], f32, tag="pp",
                                              name="pp")
                                n_mm = 3 * DC2
                                i = 0
                                for dc2 in range(DC2):
                                    # hi @ hi for the 256-chunk (both planes)
                                    nc.tensor.matmul(
                                        pp[:], w_sb[:, wi, dc2, :, 0, :],
                                        xt_t[:, dc2, :, 1, :],
                                        start=(i == 0), stop=(i == n_mm - 1),
                                        perf_mode=DR)
                                    i += 1
                                    for j in range(2):
                                        # cross: W_hi@x_lo + W_lo@x_hi
                                        nc.tensor.matmul(
                                            pp[:], w_sb[:, wi, dc2, j, :, :],
                                            xt_t[:, dc2, j, :, :],
                                            start=(i == 0),
                                            stop=(i == n_mm - 1),
                                            perf_mode=DR)
                                        i += 1
                                    if dc2 == DC2 // 2 - 1:
                                        yield
                                nc.vector.tensor_copy(
                                    dst[:, 512 * st + col0:
                                        512 * st + col0 + w_cols], pp[:])
                                yield

                        # v in [s, hd]: stationary = XT chunk, moving = W
                        pv = psA.tile([128, 4, HD], f32, tag="pp", name="pv")
                        for j4 in range(4):
                            xt_t, _, _ = parts[0]
                            jc = j4
                            n_mm = 3 * DC2
                            i = 0
                            for dc2 in range(DC2):
                                nc.tensor.matmul(
                                    pv[:, j4, :],
                                    xt_t[:, dc2, :, 1, ts(jc, 128)],
                                    w_sb[:, 2, dc2, :, 0, :],
                                    start=(i == 0), stop=(i == n_mm - 1),
                                    perf_mode=DR, skip_group_check=True)
                                i += 1
                                for j in range(2):
                                    nc.tensor.matmul(
                                        pv[:, j4, :],
                                        xt_t[:, dc2, j, :, ts(jc, 128)],
                                        w_sb[:, 2, dc2, j, :, :],
                                        start=(i == 0), stop=(i == n_mm - 1),
                                        perf_mode=DR, skip_group_check=True)
                                    i += 1
                            yield
                        nc.vector.tensor_copy(v_sb[:, ts(st, 4), :], pv[:])

                class Feeder:
                    """Queue of emission generators drained step-wise into
                    another phase's instruction stream."""

                    def __init__(self):
                        self.q = []

                    def add(self, gen):
                        self.q.append(gen)

                    def step(self, n=1):
                        for _ in range(n):
                            while self.q:
                                try:
                                    next(self.q[0])
                                    return
                                except StopIteration:
                                    self.q.pop(0)
                            return

                    def drain(self):
                        while self.q:
                            for _ in self.q.pop(0):
                                pass

                    def drain_gen(self, gen):
                        """Exhaust queued generators up to and including
                        `gen` (FIFO order)."""
                        if gen not in self.q:
                            return
                        while self.q:
                            g = self.q.pop(0)
                            for _ in g:
                                pass
                            if g is gen:
                                break

                # heads are interleaved across the two cores of a batch
                # (core parity j gets global heads j, j+2, ...).  Skip counts
                # use the SHALLOWER parity's slope so one SPMD program is
                # valid for both.
                slope_c = [0.7071067811865476 ** (2 * hh + 2)
                           for hh in range(NHG)]

                def n_skip(h, qt):
                    dist = int(30.0 / slope_c[h]) + 1
                    return max(0, (512 * qt - dist - 127) // 128 + 1)

                def emit_attn(h, qt_sb, kt_sb, v_sb, feeder=None,
                              qt_hook=None, steps=1, delay_blocks=3):
                    steep = h < 2
                    blocks_done = 0
                    for qt in range(QT_TILES):
                        if qt_hook is not None:
                            qt_hook(qt)
                        nkc = 4 * (qt + 1)
                        kc0 = n_skip(h, qt)
                        if steep:
                            shift_sb = psm.tile([128, 512], f32, tag="shift",
                                                name="shift_sb")
                            nc.sync.dma_start(
                                shift_sb[:],
                                alibi_q_in[h, ts(qt, 512)]
                                .partition_broadcast(128))
                        pot = psA.tile([128, 512], f32, tag="pot", name="pot")
                        # per-(kc, q-chunk) partial sums as single-shot psum
                        # writes (sequential start/stop groups per bank),
                        # reduced over kc on DVE afterwards
                        psums = psB.tile([128, 4, DC], f32, tag="psums",
                                         name="psums")

                        # software pipeline: stage A (scores matmul) runs one
                        # block ahead of stage B (exp + PV + sums) so the ACT
                        # exp latency hides behind the next scores matmul.
                        pst_tiles = {}

                        def stage_a(kc):
                            r = max(0, 128 * kc - 512 * qt)
                            c0 = min(r, 256)
                            pst = psST.tile([128, 512], f32, tag="pst",
                                            name="pst")
                            nc.tensor.matmul(pst[:, c0:],
                                             kt_sb[:, ts(kc, 128)],
                                             qt_sb[:, 512 * qt + c0:
                                                   512 * (qt + 1)],
                                             start=True, stop=True)
                            pst_tiles[kc] = pst

                        def stage_b(kc):
                            r = max(0, 128 * kc - 512 * qt)
                            c0 = min(r, 256)
                            pst = pst_tiles.pop(kc)
                            e_sb = pe_pool.tile([128, 512], bf16, tag="e",
                                                name="e_sb")
                            col = (h * DC + kc) * QT_TILES + qt
                            if steep:
                                t1 = pa.tile([128, 512], f32, tag="t1",
                                             name="t1")
                                nc.vector.scalar_tensor_tensor(
                                    t1[:, c0:], pst[:, c0:], PSUM_SCALE,
                                    shift_sb[:, c0:],
                                    mybir.AluOpType.mult,
                                    mybir.AluOpType.add)
                                nc.scalar.activation(
                                    e_sb[:, r:], t1[:, r:], Exp,
                                    bias=alibi_sb[:, col:col + 1],
                                    scale=1.0)
                            else:
                                nc.scalar.activation(
                                    e_sb[:, r:], pst[:, r:], Exp,
                                    bias=alibi_sb[:, col:col + 1],
                                    scale=PSUM_SCALE)
                            if r > 0:
                                nc.vector.memset(e_sb[:, :r], 0.0)
                            if kc >= 4 * qt:
                                # causal-mask the diagonal 128-band (keep
                                # j >= p).  DVE multiply by a 0/1 mask is
                                # cheaper latency than gpsimd affine_select
                                # in the exp->PV chain, but the steep heads'
                                # masked entries overflow exp (inf * 0 = nan)
                                # so they keep the select.
                                if steep:
                                    nc.gpsimd.affine_select(
                                        e_sb[:, r:r + 128],
                                        e_sb[:, r:r + 128],
                                        pattern=[[1, 128]],
                                        compare_op=mybir.AluOpType.is_ge,
                                        fill=0.0,
                                        base=0,
                                        channel_multiplier=-1)
                                else:
                                    nc.vector.tensor_mul(
                                        out=e_sb[:, r:r + 128],
                                        in0=e_sb[:, r:r + 128],
                                        in1=tri_sb[:])
                            nc.tensor.matmul(pot[:, c0:], v_sb[:, kc, :],
                                             e_sb[:, c0:],
                                             start=(kc == kc0),
                                             stop=(kc == nkc - 1))
                            # softmax sums: e as stationary, ones as moving
                            # -> [128 q-chunk, 1] columns, nearly free on PE
                            for ch in range(4):
                                nc.tensor.matmul(
                                    psums[:, ch, kc:kc + 1],
                                    e_sb[:, ts(ch, 128)],
                                    ones_col[:],
                                    start=True, stop=True,
                                    skip_group_check=True)

                        stage_a(kc0)
                        if kc0 + 1 < nkc:
                            stage_a(kc0 + 1)
                        for kc in range(kc0, nkc):
                            if kc + 2 < nkc:
                                stage_a(kc + 2)
                            if feeder is not None and blocks_done >= delay_blocks:
                                feeder.step(steps)
                            blocks_done += 1
                            stage_b(kc)

                        # 1/sums, broadcast along partitions via DRAM bounce
                        sums_sb = psm.tile([128, 4], f32, tag="sums",
                                           name="sums_sb")
                        nc.vector.tensor_reduce(
                            sums_sb[:], psums[:, :, kc0:nkc],
                            mybir.AxisListType.X, mybir.AluOpType.add)
                        recip = psm.tile([128, 4], f32, tag="recip",
                                         name="recip")
                        nc.vector.reciprocal(recip[:], sums_sb[:])
                        nc.sync.dma_start(
                            recip_dram[h, qt].rearrange(
                                "(c p) -> p c", p=128, c=4),
                            recip[:])
                        bc_sb = pa.tile([128, 512], f32, tag="t1",
                                        name="bc_sb")
                        nc.sync.dma_start(
                            bc_sb[:],
                            recip_dram[h, qt].partition_broadcast(128))
                        ot_f32 = pa.tile([128, 512], f32, tag="otf",
                                         name="ot_f32")
                        nc.vector.tensor_mul(out=ot_f32[:], in0=pot[:],
                                             in1=bc_sb[:])
                        nc.vector.tensor_copy(o_all[:, h, 1, ts(qt, 512)],
                                              ot_f32[:])
                        nc.vector.tensor_tensor(
                            out=o_all[:, h, 0, ts(qt, 512)], in0=ot_f32[:],
                            in1=o_all[:, h, 1, ts(qt, 512)],
                            op=mybir.AluOpType.subtract)

                # ---- out^T = Wo_g^T @ O^T (hi/lo fp8 DoubleRow),
                # interleaved into head 7's attention per s-tile ----
                wo_cs = []

                def load_wo():
                    for c in range(4):
                        wo_c = pwo.tile([128, NHG, 2, 512], f8, tag=f"wo{c}",
                                        name="wo_c")
                        nc.sync.dma_start(
                            wo_c[:],
                            wo_in.rearrange("p (h l f) -> p h l f",
                                            h=NHG, l=2)
                            [:, :, :, ts(c, 512)])
                        wo_cs.append(wo_c)

                def wo_mm(pp, st, mt, kind, idx, start, stop):
                    wo_c = wo_cs[mt // 4]
                    if kind == 0:
                        # hi @ hi, head-pair planes (256-deep)
                        nc.tensor.matmul(
                            pp,
                            wo_c[:, 2 * idx:2 * idx + 2, 0, ts(mt % 4, 128)],
                            o_all[:, 2 * idx:2 * idx + 2, 1, ts(st, 512)],
                            start=start, stop=stop, perf_mode=DR)
                    else:
                        # cross: Wo_hi@O_lo + Wo_lo@O_hi per head
                        nc.tensor.matmul(
                            pp,
                            wo_c[:, idx, :, ts(mt % 4, 128)],
                            o_all[:, idx, :, ts(st, 512)],
                            start=start, stop=stop, perf_mode=DR)

                # instrs not touching the last head's O (pre-emittable)
                WO_PART = ([(0, c) for c in range(3)]
                           + [(1, hh) for hh in range(NHG - 1)])
                WO_FIN = [(0, 3), (1, NHG - 1)]
                wo_open = {}

                def wo_partial_gen(st, mts):
                    """Pre-emit the head-0..6 portion of the first Wo groups
                    of tile st; groups stay open in PSUM until wo_gen."""
                    for mt in mts:
                        pp = psA.tile([128, 512], f32, tag="pp", name="pp")
                        for i, (k, idx) in enumerate(WO_PART):
                            wo_mm(pp[:], st, mt, k, idx, i == 0, False)
                        wo_open[(st, mt)] = pp
                        yield

                def wo_gen(st):
                    for mt in range(D // 128):
                        pp = wo_open.pop((st, mt), None)
                        if pp is not None:
                            for i, (k, idx) in enumerate(WO_FIN):
                                wo_mm(pp[:], st, mt, k, idx, False,
                                      i == len(WO_FIN) - 1)
                        else:
                            pp = psA.tile([128, 512], f32, tag="pp",
                                          name="pp")
                            seq = WO_PART + WO_FIN
                            for i, (k, idx) in enumerate(seq):
                                wo_mm(pp[:], st, mt, k, idx, i == 0,
                                      i == len(seq) - 1)
                        o_sb = post.tile([128, 512], f32, tag="osb",
                                         name="o_sb")
                        nc.scalar.mul(o_sb[:], pp[:], 1.0 / WS)
                        nc.sync.dma_start(outT[ts(mt, 128), ts(st, 512)],
                                          o_sb[:])
                        yield

                # Head emission order: shallow heads first, steepest last —
                # the final attention (which has nothing left to interleave
                # except the qt-gated Wo stage) is then the shortest.
                # Weights are loaded two heads ahead so interleaved proj
                # chunks never wait on their weight DMA.
                # DMA order at t=0: wq(first head), XT quarter-0 halves, wk,
                # wv, XT quarters 1-3.
                perm = list(range(NHG))
                w_map = {}
                # PE warm-up: back-to-back dummy matmuls on a zeroed
                # tile keep the PE continuously busy through the initial
                # wq+XT0 DMA lead-in, so the p-state ramp (0.65->1.2->2.4
                # GHz after 3us of continuous execution) completes before
                # the first real projection matmul.
                warm = psm.tile([128, 512], bf16, tag="warm", name="warm")
                nc.vector.memset(warm[:], 0.0)
                for wi_ in range(30):
                    wps = psST.tile([128, 512], f32, tag="pst", name="wps")
                    nc.tensor.matmul(wps[:], warm[:, :128], warm[:],
                                     start=True, stop=True)

                w_map[perm[0]] = load_w(perm[0],
                                        after_wq=lambda: load_xt([0]))
                load_xt([1])
                load_consts()
                load_xt([2, 3])
                w_map[perm[1]] = load_w(perm[1])

                holds = {}
                gens = {}
                f = Feeder()

                def add_proj(i):
                    h = perm[i]
                    holds[h] = {}
                    gens[h] = proj_gen(h, w_map[h], holds[h])
                    f.add(gens[h])

                add_proj(0)
                f.drain()
                add_proj(1)
                for i in range(NHG):
                    h = perm[i]
                    qt_hook = None
                    steps = 1
                    if i + 2 < NHG:
                        w_map[perm[i + 2]] = load_w(perm[i + 2])
                    if i + 2 == NHG:
                        load_wo()
                    if i == NHG - 1:
                        steps = 2

                        def qt_hook(qt, _f=f):
                            if qt == 0:
                                _f.add(wo_partial_gen(0, [0, 1]))
                            else:
                                _f.add(wo_gen(qt - 1))
                    emit_attn(h, holds[h]["q"], holds[h]["k"], holds[h]["v"],
                              feeder=f, qt_hook=qt_hook, steps=steps,
                              delay_blocks=0)
                    if i + 1 < NHG:
                        f.drain_gen(gens[perm[i + 1]])
                        # only now start the next-next head's projection so
                        # at most two QT/KT/V generations are ever live
                        if i + 2 < NHG:
                            add_proj(i + 2)
                f.drain()
                f.add(wo_gen(QT_TILES - 1))
                f.drain()

    nc.compile()
    return nc


def _hilo(t):
    import ml_dtypes
    hi = t.astype(ml_dtypes.float8_e4m3)
    lo = (t - hi.astype(np.float32)).astype(ml_dtypes.float8_e4m3)
    return hi, lo


def _pack_xt(xb):
    """x[b] [S, D] fp32 -> [128, DC2*2*2*S] fp8; layout [p, dc2, c, hl(lo,hi), s]
    with d = dc2*256 + c*128 + p."""
    xt = np.ascontiguousarray(xb.T)                     # [D, S]
    hi, lo = _hilo(xt)
    arr = np.stack([lo, hi], axis=1)                    # [D, 2, S]
    arr = arr.reshape(DC2, 2, 128, 2, S)                # [dc2, c, p, hl, s]
    arr = arr.transpose(2, 0, 1, 3, 4)                  # [p, dc2, c, hl, s]
    return np.ascontiguousarray(arr).reshape(128, -1)


def _pack_wo(Wo, heads):
    """Wo [D, D] fp32 -> [128, NHG*2*D] fp8; layout [p, h, hl(hi,lo), f];
    row m = h*HD + p, scaled by WS."""
    rows = np.concatenate([Wo[h * HD:(h + 1) * HD, :] for h in heads], axis=0)
    hi, lo = _hilo(rows * np.float32(WS))               # [NHG*HD, D]
    arr = np.stack([hi, lo], axis=1)                    # [NHG*HD, 2, D]
    arr = arr.reshape(NHG, 128, 2, D)                   # [h, p, hl, f]
    arr = arr.transpose(1, 0, 2, 3)                     # [p, h, hl, f]
    return np.ascontiguousarray(arr).reshape(128, -1)


def _pack_w(w, heads):
    """W [D, D] fp32 -> [128, NHG*DC2*2*2*HD] fp8 for the given head list;
    layout [p, h, dc2, c, hl(hi,lo), hd], d = dc2*256 + c*128 + p, scaled WS."""
    cols = np.concatenate([w[:, h * HD:(h + 1) * HD] for h in heads], axis=1)
    hi, lo = _hilo(cols * np.float32(WS))               # [D, NHG*HD]
    arr = np.stack([hi, lo], axis=1)                    # [D, 2, NHG*HD]
    arr = arr.reshape(DC2, 2, 128, 2, NHG, HD)          # [dc2, c, p, hl, h, hd]
    arr = arr.transpose(2, 4, 0, 1, 3, 5)               # [p, h, dc2, c, hl, hd]
    return np.ascontiguousarray(arr).reshape(128, -1)


def _in_maps(x, Wq, Wk, Wv, Wo):
    import ml_dtypes
    slopes = np.asarray(_get_slopes(NH), dtype=np.float32)
    pos = np.arange(S, dtype=np.float32)
    dist = np.float32(S - 1) - pos                       # (S,)
    ones_col = np.full((128, 1), WS, ml_dtypes.bfloat16)
    tri = np.triu(np.ones((128, 128), np.float32)).astype(ml_dtypes.bfloat16)

    xt_packed = [_pack_xt(x[b]) for b in range(B)]
    w_packed = {}
    for g in range(2):
        heads = list(range(g, NH, 2))
        w_packed[g] = (
            _pack_w(Wq, heads), _pack_w(Wk, heads), _pack_w(Wv, heads),
            _pack_wo(Wo, heads))

    in_maps = []
    for b in range(B):
        for g in range(2):
            heads = list(range(g, NH, 2))
            sl = slopes[heads]                            # (8,)
            ab = np.empty((128, NHG * DC * QT_TILES), np.float32)
            d2 = dist.reshape(DC, 128)                    # [kc, p]
            for h in range(NHG):
                for kc in range(DC):
                    a_col = (-sl[h] * d2[kc]).astype(np.float32)  # (128,)
                    for qt in range(QT_TILES):
                        if h < 2:
                            c = np.float32(0.0)
                        else:
                            q_mid = 512 * qt + 255.5
                            c = np.float32(sl[h] * (S - 1 - q_mid))
                        ab[:, (h * DC + kc) * QT_TILES + qt] = a_col + c
            alibi_q = (sl[:, None] * dist[None, :]).astype(np.float32)
            wq8, wk8, wv8, wo8 = w_packed[g]
            in_maps.append({
                "xt8": xt_packed[b],
                "wq8": wq8,
                "wk8": wk8,
                "wv8": wv8,
                "wo8": wo8,
                "alibi_b": ab,
                "alibi_q": alibi_q,
                "ones_col": ones_col,
                "tri": tri,
            })
    return in_maps


def kernel(x, Wq, Wk, Wv, Wo, _trace=False):
    from concourse.bass_utils import run_bass_kernel_spmd

    if "nc" not in _cache:
        _cache["nc"] = _build()
    nc = _cache["nc"]

    res = run_bass_kernel_spmd(
        nc, _in_maps(x, Wq, Wk, Wv, Wo), core_ids=list(range(2 * B)),
        trace=_trace)
    _cache["last_exec_time_ns"] = res.exec_time_ns

    out = np.empty((B, S, D), dtype=np.float32)
    for b in range(B):
        out[b] = (res.results[2 * b]["outT"] + res.results[2 * b + 1]["outT"]).T
    return out



# revision 3
# speedup vs baseline: 1.2123x; 1.2123x over previous
"""Causal attention with ALiBi for Trainium2, tensor-parallel over heads x
data-parallel over batch (8 NeuronCores).

Problem: B=4, S=2048, D=2048, NH=16, HD=128, fp32.
  q/k/v = x @ Wq/Wk/Wv ; scores = q k^T / sqrt(HD) + alibi ; causal softmax ;
  out = (probs @ v) @ Wo

Sharding: core (b, j) handles batch b and the 8 interleaved heads
  j, j+2, ..., j+14.  Each core returns out_partial^T; the host sums the two
  per-batch partials and transposes back.

Implementation notes (v2):
  * x^T is built on the HOST and shipped as hi/lo fp8e4 pairs (x = hi + lo,
    lo = fp8(x - hi)); Wq/Wk/Wv likewise (scaled by 128 to stay in the fp8
    normal range).  QKV projections run as fp8 DoubleRow matmuls (2 k-planes
    per instruction, 256-deep contraction at 157 TF/s):
      q = x_hi@W_hi  (8 instrs, chunk pairs)
        + x_lo@W_hi + x_hi@W_lo  (16 instrs, one per 128-chunk, the two
          cross terms paired as the two planes of one DoubleRow instr)
    The lo*lo term (~2^-8 relative) is dropped; projection accuracy ~bf16.
  * V is produced directly in [s, hd] layout (XT as the stationary operand),
    so there are no PE transposes anywhere.
  * Attention interior is bf16: scores = K^T Q per [128k x 512q] block,
    exp on ACT (bias column carries the ALiBi + per-q-tile shift constant;
    the two steepest local heads apply an exact per-q shift on DVE first),
    P@V accumulated in PSUM.
  * Softmax sums use e_sb as the matmul STATIONARY with a ones column as
    the moving operand: out [128q-chunk, 1] per chunk, accumulated in PSUM
    across k-blocks.  The ones column carries the value 128 so the V scale
    (weights pre-scaled by 128) cancels in pot/sums.
    The per-(h,qt) reciprocal row is broadcast across partitions via a DRAM
    bounce (SBUF[128,4] -> DRAM[512] -> partition_broadcast DMA).
  * O stays in SBUF (bf16, all heads); out^T = Wo^T @ O^T in bf16.
  * ALiBi block skipping as in v1 (blocks with decay < ~e^-18 of the softmax
    sum are skipped; fully-masked prefixes of diagonal blocks trimmed;
    partial bands masked by a DVE triangular-mask multiply, except the two
    steepest local heads where masked entries overflow exp and a gpsimd
    affine_select is used instead).
  * Scheduling: x^T arrives in four s-quarter DMAs so the first projection
    group starts ~7us in; weights are prefetched two heads ahead; each
    head's projection is EMITTED INTERLEAVED into the previous head's
    attention block loop (a generator-feeder), filling the ACT exp latency
    bubbles on the PE; the Wo stage is interleaved into the last head's
    attention per s-tile.  Measured (TimelineSim): ~451 us vs 735 us for
    the f32r v1 baseline, PE ~93% busy.
"""

import math

import numpy as np

B, S, D, NH = 4, 2048, 2048, 16
HD = D // NH            # 128
NHG = NH // 2           # heads per core
DC = D // 128           # 16 d-chunks
DC2 = DC // 2           # 8 256-wide d-chunks
QT_TILES = S // 512     # 4 q tiles
SCALE = 1.0 / math.sqrt(HD)
WS = 128.0              # weight pre-scale for fp8
PSUM_SCALE = SCALE / (WS * WS)   # q,k both carry a factor WS

_cache = {}


def _get_slopes(n):
    def pow2(n):
        start = 2 ** (-(2 ** (-(math.log2(n) - 3))))
        return [start * start**i for i in range(n)]

    if math.log2(n).is_integer():
        return pow2(n)
    c = 2 ** math.floor(math.log2(n))
    return pow2(c) + _get_slopes(2 * c)[0::2][: n - c]


def _build():
    import concourse.bacc as bacc
    import concourse.mybir as mybir
    import concourse.tile as tile
    from concourse.bass import ts

    f32 = mybir.dt.float32
    bf16 = mybir.dt.bfloat16
    f8 = mybir.dt.float8e4
    Exp = mybir.ActivationFunctionType.Exp
    DR = mybir.MatmulPerfMode.DoubleRow

    nc = bacc.Bacc()
    # xt8[p, ((dc2*2 + c)*2 + hl)*S + s]; d = dc2*256 + c*128 + p; hl=(lo,hi)
    xt_in = nc.declare_dram_parameter("xt8", [128, DC2 * 2 * 2 * S], f8,
                                      isOutput=False)
    # w8[p, (((h*DC2 + dc2)*2 + c)*2 + hl)*HD + hd]; hl=(hi,lo)
    wq_in = nc.declare_dram_parameter("wq8", [128, NHG * DC2 * 2 * 2 * HD], f8,
                                      isOutput=False)
    wk_in = nc.declare_dram_parameter("wk8", [128, NHG * DC2 * 2 * 2 * HD], f8,
                                      isOutput=False)
    wv_in = nc.declare_dram_parameter("wv8", [128, NHG * DC2 * 2 * 2 * HD], f8,
                                      isOutput=False)
    # wo8[p, (h*2 + hl)*D + f]; row m = h*HD + p; hl=(hi,lo); scaled by WS
    wo_in = nc.declare_dram_parameter("wo8", [128, NHG * 2 * D], f8,
                                      isOutput=False)
    # alibi_b[p, ((h*16+kc)*4+qt)] = -slope_h*(S-1-(kc*128+p)) + C[h,qt]
    alibi_b_in = nc.declare_dram_parameter(
        "alibi_b", [128, NHG * DC * QT_TILES], f32, isOutput=False)
    # alibi_q[h, q] = +slope_h * (S-1 - q)   (per-query shift)
    alibi_q_in = nc.declare_dram_parameter("alibi_q", [NHG, S], f32,
                                           isOutput=False)
    ones_col_in = nc.declare_dram_parameter("ones_col", [128, 1], bf16,
                                            isOutput=False)
    # tri[p, j] = 1 if j >= p else 0 — causal mask for the diagonal
    # 128-band of every score block (local coords are the same for all)
    tri_in = nc.declare_dram_parameter("tri", [128, 128], bf16,
                                       isOutput=False)
    outT = nc.declare_dram_parameter("outT", [D, S], f32, isOutput=True)

    recip_dram = nc.dram_tensor("recip_scratch", [NHG, QT_TILES, 512], f32)

    with tile.TileContext(nc) as tc:
        with (
            tc.tile_pool(name="consts", bufs=1) as pc,
            tc.tile_pool(name="psA", bufs=2, space="PSUM") as psA,
            tc.tile_pool(name="psST", bufs=3, space="PSUM") as psST,
            tc.tile_pool(name="psB", bufs=1, space="PSUM") as psB,
        ):
            alibi_sb = pc.tile([128, NHG * DC * QT_TILES], f32,
                               name="alibi_sb")
            ones_col = pc.tile([128, 1], bf16, name="ones_col_sb")
            tri_sb = pc.tile([128, 128], bf16, name="tri_sb")
            # O as hi/lo fp8 pairs for the DoubleRow Wo stage; hl=(lo,hi)
            o_all = pc.tile([128, NHG, 2, S], f8, name="o_all")

            def load_consts():
                nc.sync.dma_start(alibi_sb[:], alibi_b_in[:])
                nc.sync.dma_start(ones_col[:], ones_col_in[:])
                nc.sync.dma_start(tri_sb[:], tri_in[:])

            with (
                tc.tile_pool(name="xt", bufs=1) as pxt,
                tc.tile_pool(name="wp", bufs=2) as pw,
                tc.tile_pool(name="qkv2", bufs=2) as pq2,
                tc.tile_pool(name="qkv", bufs=2) as pq,
                tc.tile_pool(name="att", bufs=2) as pa,
                tc.tile_pool(name="epool", bufs=4) as pe_pool,
                tc.tile_pool(name="small", bufs=2) as psm,
                tc.tile_pool(name="wo", bufs=1) as pwo,
                tc.tile_pool(name="ost", bufs=4) as post,
            ):
                # XT8[p, dc2, c, hl, s-slice] per s-quarter (hl: 0=lo, 1=hi);
                # one tile per 512-wide s-quarter so the first projection
                # group (st=0) only waits for the first DMA, and the proj
                # pipeline keeps pace with the arrivals.  Emitted after the
                # first head's (small) weight DMA, see below.
                XTq = [None] * QT_TILES

                def load_xt(quarters):
                    xin = xt_in.rearrange("p (a c l s) -> p a c l s",
                                          a=DC2, c=2, l=2)
                    for sq in quarters:
                        xt_t = pxt.tile([128, DC2, 2, 2, 512], f8,
                                        tag=f"xt{sq}", name=f"XT{sq}")
                        nc.sync.dma_start(
                            xt_t[:], xin[:, :, :, :, ts(sq, 512)])
                        XTq[sq] = xt_t

                def load_w(h, after_wq=None):
                    w_sb = pw.tile([128, 3, DC2, 2, 2, HD], f8, tag="w",
                                   name="w_sb")
                    for wi, w_in in enumerate((wq_in, wk_in, wv_in)):
                        nc.sync.dma_start(
                            w_sb[:, wi],
                            w_in[:, ts(h, DC2 * 2 * 2 * HD)].rearrange(
                                "p (a c l f) -> p a c l f", a=DC2, c=2, l=2))
                        if wi == 0 and after_wq is not None:
                            after_wq()
                    return w_sb

                def proj_gen(h, w_sb, out):
                    """Generator projecting q, k (as [hd, S] bf16) and v
                    ([s-chunk, sc, hd] bf16) for local head h via hi/lo fp8
                    DoubleRow.  Yields every ~12 matmuls so the driver can
                    interleave the emission into the previous head's
                    attention (filling ACT-latency bubbles on the PE)."""
                    qt_sb = pq2.tile([128, S], bf16, tag="QT", name="qt_sb")
                    kt_sb = pq2.tile([128, S], bf16, tag="KT", name="kt_sb")
                    v_sb = pq.tile([128, DC, HD], bf16, tag="V", name="v_sb")
                    out["q"], out["k"], out["v"] = qt_sb, kt_sb, v_sb

                    for st in range(QT_TILES):
                        parts = [(XTq[st], 0, 512)]
                        # q^T, k^T: stationary = W chunk, moving = XT.
                        # Pure fp8 (hi @ hi only): the ~2^-4 relative noise
                        # on q,k perturbs scores by ~0.02 abs -> ~2% on
                        # probs, well inside the 2e-2 output budget.
                        for wi, dst in ((0, qt_sb), (1, kt_sb)):
                            for xt_t, col0, w_cols in parts:
                                pp = psA.tile([128, w_cols], f32, tag="pp",
                                              name="pp")
                                n_mm = DC2
                                for dc2 in range(DC2):
                                    # hi @ hi for the 256-chunk (both planes)
                                    nc.tensor.matmul(
                                        pp[:], w_sb[:, wi, dc2, :, 0, :],
                                        xt_t[:, dc2, :, 1, :],
                                        start=(dc2 == 0),
                                        stop=(dc2 == n_mm - 1),
                                        perf_mode=DR)
                                    if dc2 == DC2 // 2 - 1:
                                        yield
                                nc.vector.tensor_copy(
                                    dst[:, 512 * st + col0:
                                        512 * st + col0 + w_cols], pp[:])
                                yield

                        # v in [s, hd]: stationary = XT chunk, moving = W
                        pv = psA.tile([128, 4, HD], f32, tag="pp", name="pv")
                        for j4 in range(4):
                            xt_t, _, _ = parts[0]
                            jc = j4
                            n_mm = 3 * DC2
                            i = 0
                            for dc2 in range(DC2):
                                nc.tensor.matmul(
                                    pv[:, j4, :],
                                    xt_t[:, dc2, :, 1, ts(jc, 128)],
                                    w_sb[:, 2, dc2, :, 0, :],
                                    start=(i == 0), stop=(i == n_mm - 1),
                                    perf_mode=DR, skip_group_check=True)
                                i += 1
                                for j in range(2):
                                    nc.tensor.matmul(
                                        pv[:, j4, :],
                                        xt_t[:, dc2, j, :, ts(jc, 128)],
                                        w_sb[:, 2, dc2, j, :, :],
                                        start=(i == 0), stop=(i == n_mm - 1),
                                        perf_mode=DR, skip_group_check=True)
                                    i += 1
                            yield
                        nc.vector.tensor_copy(v_sb[:, ts(st, 4), :], pv[:])

                class Feeder:
                    """Queue of emission generators drained step-wise into
                    another phase's instruction stream."""

                    def __init__(self):
                        self.q = []

                    def add(self, gen):
                        self.q.append(gen)

                    def step(self, n=1):
                        for _ in range(n):
                            while self.q:
                                try:
                                    next(self.q[0])
                                    return
                                except StopIteration:
                                    self.q.pop(0)
                            return

                    def drain(self):
                        while self.q:
                            for _ in self.q.pop(0):
                                pass

                    def drain_gen(self, gen):
                        """Exhaust queued generators up to and including
                        `gen` (FIFO order)."""
                        if gen not in self.q:
                            return
                        while self.q:
                            g = self.q.pop(0)
                            for _ in g:
                                pass
                            if g is gen:
                                break

                # heads are interleaved across the two cores of a batch
                # (core parity j gets global heads j, j+2, ...).  Skip counts
                # use the SHALLOWER parity's slope so one SPMD program is
                # valid for both.
                slope_c = [0.7071067811865476 ** (2 * hh + 2)
                           for hh in range(NHG)]

                def n_skip(h, qt):
                    # skipped softmax mass per query ~= e^-T / slope; T=12
                    # keeps it under ~4e-4 of the sum for every head.
                    dist = int(12.0 / slope_c[h]) + 1
                    return max(0, (512 * qt - dist - 127) // 128 + 1)

                def emit_attn(h, qt_sb, kt_sb, v_sb, feeder=None,
                              qt_hook=None, steps=1, delay_blocks=3):
                    steep = h < 2
                    blocks_done = 0
                    for qt in range(QT_TILES):
                        if qt_hook is not None:
                            qt_hook(qt)
                        nkc = 4 * (qt + 1)
                        kc0 = n_skip(h, qt)
                        if steep:
                            shift_sb = psm.tile([128, 512], f32, tag="shift",
                                                name="shift_sb")
                            nc.sync.dma_start(
                                shift_sb[:],
                                alibi_q_in[h, ts(qt, 512)]
                                .partition_broadcast(128))
                        pot = psA.tile([128, 512], f32, tag="pot", name="pot")
                        # per-(kc, q-chunk) partial sums as single-shot psum
                        # writes (sequential start/stop groups per bank),
                        # reduced over kc on DVE afterwards
                        psums = psB.tile([128, 4, DC], f32, tag="psums",
                                         name="psums")

                        # software pipeline: stage A (scores matmul) runs one
                        # block ahead of stage B (exp + PV + sums) so the ACT
                        # exp latency hides behind the next scores matmul.
                        pst_tiles = {}

                        def stage_a(kc):
                            r = max(0, 128 * kc - 512 * qt)
                            c0 = min(r, 256)
                            pst = psST.tile([128, 512], f32, tag="pst",
                                            name="pst")
                            nc.tensor.matmul(pst[:, c0:],
                                             kt_sb[:, ts(kc, 128)],
                                             qt_sb[:, 512 * qt + c0:
                                                   512 * (qt + 1)],
                                             start=True, stop=True)
                            pst_tiles[kc] = pst

                        def stage_b(kc):
                            r = max(0, 128 * kc - 512 * qt)
                            c0 = min(r, 256)
                            pst = pst_tiles.pop(kc)
                            e_sb = pe_pool.tile([128, 512], bf16, tag="e",
                                                name="e_sb")
                            col = (h * DC + kc) * QT_TILES + qt
                            if steep:
                                t1 = pa.tile([128, 512], f32, tag="t1",
                                             name="t1")
                                nc.vector.scalar_tensor_tensor(
                                    t1[:, c0:], pst[:, c0:], PSUM_SCALE,
                                    shift_sb[:, c0:],
                                    mybir.AluOpType.mult,
                                    mybir.AluOpType.add)
                                nc.scalar.activation(
                                    e_sb[:, r:], t1[:, r:], Exp,
                                    bias=alibi_sb[:, col:col + 1],
                                    scale=1.0)
                            else:
                                nc.scalar.activation(
                                    e_sb[:, r:], pst[:, r:], Exp,
                                    bias=alibi_sb[:, col:col + 1],
                                    scale=PSUM_SCALE)
                            if r > 0:
                                nc.vector.memset(e_sb[:, :r], 0.0)
                            if kc >= 4 * qt:
                                # causal-mask the diagonal 128-band (keep
                                # j >= p).  DVE multiply by a 0/1 mask is
                                # cheaper latency than gpsimd affine_select
                                # in the exp->PV chain, but the steep heads'
                                # masked entries overflow exp (inf * 0 = nan)
                                # so they keep the select.
                                if steep:
                                    nc.gpsimd.affine_select(
                                        e_sb[:, r:r + 128],
                                        e_sb[:, r:r + 128],
                                        pattern=[[1, 128]],
                                        compare_op=mybir.AluOpType.is_ge,
                                        fill=0.0,
                                        base=0,
                                        channel_multiplier=-1)
                                else:
                                    nc.vector.tensor_mul(
                                        out=e_sb[:, r:r + 128],
                                        in0=e_sb[:, r:r + 128],
                                        in1=tri_sb[:])
                            nc.tensor.matmul(pot[:, c0:], v_sb[:, kc, :],
                                             e_sb[:, c0:],
                                             start=(kc == kc0),
                                             stop=(kc == nkc - 1))
                            # softmax sums: e as stationary, ones as moving
                            # -> [128 q-chunk, 1] columns, nearly free on PE
                            for ch in range(4):
                                nc.tensor.matmul(
                                    psums[:, ch, kc:kc + 1],
                                    e_sb[:, ts(ch, 128)],
                                    ones_col[:],
                                    start=True, stop=True,
                                    skip_group_check=True)

                        stage_a(kc0)
                        if kc0 + 1 < nkc:
                            stage_a(kc0 + 1)
                        for kc in range(kc0, nkc):
                            if kc + 2 < nkc:
                                stage_a(kc + 2)
                            if feeder is not None and blocks_done >= delay_blocks:
                                feeder.step(steps)
                            blocks_done += 1
                            stage_b(kc)

                        # 1/sums, broadcast along partitions via DRAM bounce
                        sums_sb = psm.tile([128, 4], f32, tag="sums",
                                           name="sums_sb")
                        nc.vector.tensor_reduce(
                            sums_sb[:], psums[:, :, kc0:nkc],
                            mybir.AxisListType.X, mybir.AluOpType.add)
                        recip = psm.tile([128, 4], f32, tag="recip",
                                         name="recip")
                        nc.vector.reciprocal(recip[:], sums_sb[:])
                        nc.sync.dma_start(
                            recip_dram[h, qt].rearrange(
                                "(c p) -> p c", p=128, c=4),
                            recip[:])
                        bc_sb = pa.tile([128, 512], f32, tag="t1",
                                        name="bc_sb")
                        nc.sync.dma_start(
                            bc_sb[:],
                            recip_dram[h, qt].partition_broadcast(128))
                        ot_f32 = pa.tile([128, 512], f32, tag="otf",
                                         name="ot_f32")
                        nc.vector.tensor_mul(out=ot_f32[:], in0=pot[:],
                                             in1=bc_sb[:])
                        nc.vector.tensor_copy(o_all[:, h, 1, ts(qt, 512)],
                                              ot_f32[:])
                        nc.vector.tensor_tensor(
                            out=o_all[:, h, 0, ts(qt, 512)], in0=ot_f32[:],
                            in1=o_all[:, h, 1, ts(qt, 512)],
                            op=mybir.AluOpType.subtract)

                # ---- out^T = Wo_g^T @ O^T (hi/lo fp8 DoubleRow),
                # interleaved into head 7's attention per s-tile ----
                wo_cs = []

                def load_wo():
                    for c in range(4):
                        wo_c = pwo.tile([128, NHG, 2, 512], f8, tag=f"wo{c}",
                                        name="wo_c")
                        nc.sync.dma_start(
                            wo_c[:],
                            wo_in.rearrange("p (h l f) -> p h l f",
                                            h=NHG, l=2)
                            [:, :, :, ts(c, 512)])
                        wo_cs.append(wo_c)

                def wo_mm(pp, st, mt, kind, idx, start, stop):
                    wo_c = wo_cs[mt // 4]
                    if kind == 0:
                        # hi @ hi, head-pair planes (256-deep)
                        nc.tensor.matmul(
                            pp,
                            wo_c[:, 2 * idx:2 * idx + 2, 0, ts(mt % 4, 128)],
                            o_all[:, 2 * idx:2 * idx + 2, 1, ts(st, 512)],
                            start=start, stop=stop, perf_mode=DR)
                    else:
                        # cross: Wo_hi@O_lo + Wo_lo@O_hi per head
                        nc.tensor.matmul(
                            pp,
                            wo_c[:, idx, :, ts(mt % 4, 128)],
                            o_all[:, idx, :, ts(st, 512)],
                            start=start, stop=stop, perf_mode=DR)

                # instrs not touching the last head's O (pre-emittable)
                WO_PART = ([(0, c) for c in range(3)]
                           + [(1, hh) for hh in range(NHG - 1)])
                WO_FIN = [(0, 3), (1, NHG - 1)]
                wo_open = {}

                def wo_partial_gen(st, mts):
                    """Pre-emit the head-0..6 portion of the first Wo groups
                    of tile st; groups stay open in PSUM until wo_gen."""
                    for mt in mts:
                        pp = psA.tile([128, 512], f32, tag="pp", name="pp")
                        for i, (k, idx) in enumerate(WO_PART):
                            wo_mm(pp[:], st, mt, k, idx, i == 0, False)
                        wo_open[(st, mt)] = pp
                        yield

                def wo_gen(st):
                    for mt in range(D // 128):
                        pp = wo_open.pop((st, mt), None)
                        if pp is not None:
                            for i, (k, idx) in enumerate(WO_FIN):
                                wo_mm(pp[:], st, mt, k, idx, False,
                                      i == len(WO_FIN) - 1)
                        else:
                            pp = psA.tile([128, 512], f32, tag="pp",
                                          name="pp")
                            seq = WO_PART + WO_FIN
                            for i, (k, idx) in enumerate(seq):
                                wo_mm(pp[:], st, mt, k, idx, i == 0,
                                      i == len(seq) - 1)
                        o_sb = post.tile([128, 512], f32, tag="osb",
                                         name="o_sb")
                        nc.scalar.mul(o_sb[:], pp[:], 1.0 / WS)
                        nc.sync.dma_start(outT[ts(mt, 128), ts(st, 512)],
                                          o_sb[:])
                        yield

                # Head emission order: shallow heads first, steepest last —
                # the final attention (which has nothing left to interleave
                # except the qt-gated Wo stage) is then the shortest.
                # Weights are loaded two heads ahead so interleaved proj
                # chunks never wait on their weight DMA.
                # DMA order at t=0: wq(first head), XT quarter-0 halves, wk,
                # wv, XT quarters 1-3.
                perm = list(range(NHG))
                w_map = {}
                # PE warm-up: back-to-back dummy matmuls on a zeroed
                # tile keep the PE continuously busy through the initial
                # wq+XT0 DMA lead-in, so the p-state ramp (0.65->1.2->2.4
                # GHz after 3us of continuous execution) completes before
                # the first real projection matmul.
                warm = psm.tile([128, 512], bf16, tag="warm", name="warm")
                nc.vector.memset(warm[:], 0.0)
                for wi_ in range(30):
                    wps = psST.tile([128, 512], f32, tag="pst", name="wps")
                    nc.tensor.matmul(wps[:], warm[:, :128], warm[:],
                                     start=True, stop=True)

                w_map[perm[0]] = load_w(perm[0],
                                        after_wq=lambda: load_xt([0]))
                load_xt([1])
                load_consts()
                load_xt([2, 3])
                w_map[perm[1]] = load_w(perm[1])

                holds = {}
                gens = {}
                f = Feeder()

                def add_proj(i):
                    h = perm[i]
                    holds[h] = {}
                    gens[h] = proj_gen(h, w_map[h], holds[h])
                    f.add(gens[h])

                add_proj(0)
                f.drain()
                add_proj(1)
                for i in range(NHG):
                    h = perm[i]
                    qt_hook = None
                    steps = 1
                    if i + 2 < NHG:
                        w_map[perm[i + 2]] = load_w(perm[i + 2])
                    if i + 2 == NHG:
                        load_wo()
                    if i == NHG - 1:
                        steps = 2

                        def qt_hook(qt, _f=f):
                            if qt == 0:
                                _f.add(wo_partial_gen(0, [0, 1]))
                            else:
                                _f.add(wo_gen(qt - 1))
                    emit_attn(h, holds[h]["q"], holds[h]["k"], holds[h]["v"],
                              feeder=f, qt_hook=qt_hook, steps=steps,
                              delay_blocks=0)
                    if i + 1 < NHG:
                        f.drain_gen(gens[perm[i + 1]])
                        # only now start the next-next head's projection so
                        # at most two QT/KT/V generations are ever live
                        if i + 2 < NHG:
                            add_proj(i + 2)
                f.drain()
                f.add(wo_gen(QT_TILES - 1))
                f.drain()

    nc.compile()
    return nc


def _hilo(t):
    import ml_dtypes
    hi = t.astype(ml_dtypes.float8_e4m3)
    lo = (t - hi.astype(np.float32)).astype(ml_dtypes.float8_e4m3)
    return hi, lo


def _pack_xt(xb):
    """x[b] [S, D] fp32 -> [128, DC2*2*2*S] fp8; layout [p, dc2, c, hl(lo,hi), s]
    with d = dc2*256 + c*128 + p."""
    xt = np.ascontiguousarray(xb.T)                     # [D, S]
    hi, lo = _hilo(xt)
    arr = np.stack([lo, hi], axis=1)                    # [D, 2, S]
    arr = arr.reshape(DC2, 2, 128, 2, S)                # [dc2, c, p, hl, s]
    arr = arr.transpose(2, 0, 1, 3, 4)                  # [p, dc2, c, hl, s]
    return np.ascontiguousarray(arr).reshape(128, -1)


def _pack_wo(Wo, heads):
    """Wo [D, D] fp32 -> [128, NHG*2*D] fp8; layout [p, h, hl(hi,lo), f];
    row m = h*HD + p, scaled by WS."""
    rows = np.concatenate([Wo[h * HD:(h + 1) * HD, :] for h in heads], axis=0)
    hi, lo = _hilo(rows * np.float32(WS))               # [NHG*HD, D]
    arr = np.stack([hi, lo], axis=1)                    # [NHG*HD, 2, D]
    arr = arr.reshape(NHG, 128, 2, D)                   # [h, p, hl, f]
    arr = arr.transpose(1, 0, 2, 3)                     # [p, h, hl, f]
    return np.ascontiguousarray(arr).reshape(128, -1)


def _pack_w(w, heads):
    """W [D, D] fp32 -> [128, NHG*DC2*2*2*HD] fp8 for the given head list;
    layout [p, h, dc2, c, hl(hi,lo), hd], d = dc2*256 + c*128 + p, scaled WS."""
    cols = np.concatenate([w[:, h * HD:(h + 1) * HD] for h in heads], axis=1)
    hi, lo = _hilo(cols * np.float32(WS))               # [D, NHG*HD]
    arr = np.stack([hi, lo], axis=1)                    # [D, 2, NHG*HD]
    arr = arr.reshape(DC2, 2, 128, 2, NHG, HD)          # [dc2, c, p, hl, h, hd]
    arr = arr.transpose(2, 4, 0, 1, 3, 5)               # [p, h, dc2, c, hl, hd]
    return np.ascontiguousarray(arr).reshape(128, -1)


def _in_maps(x, Wq, Wk, Wv, Wo):
    import ml_dtypes
    slopes = np.asarray(_get_slopes(NH), dtype=np.float32)
    pos = np.arange(S, dtype=np.float32)
    dist = np.float32(S - 1) - pos                       # (S,)
    ones_col = np.full((128, 1), WS, ml_dtypes.bfloat16)
    tri = np.triu(np.ones((128, 128), np.float32)).astype(ml_dtypes.bfloat16)

    xt_packed = [_pack_xt(x[b]) for b in range(B)]
    w_packed = {}
    for g in range(2):
        heads = list(range(g, NH, 2))
        w_packed[g] = (
            _pack_w(Wq, heads), _pack_w(Wk, heads), _pack_w(Wv, heads),
            _pack_wo(Wo, heads))

    in_maps = []
    for b in range(B):
        for g in range(2):
            heads = list(range(g, NH, 2))
            sl = slopes[heads]                            # (8,)
            ab = np.empty((128, NHG * DC * QT_TILES), np.float32)
            d2 = dist.reshape(DC, 128)                    # [kc, p]
            for h in range(NHG):
                for kc in range(DC):
                    a_col = (-sl[h] * d2[kc]).astype(np.float32)  # (128,)
                    for qt in range(QT_TILES):
                        if h < 2:
                            c = np.float32(0.0)
                        else:
                            q_mid = 512 * qt + 255.5
                            c = np.float32(sl[h] * (S - 1 - q_mid))
                        ab[:, (h * DC + kc) * QT_TILES + qt] = a_col + c
            alibi_q = (sl[:, None] * dist[None, :]).astype(np.float32)
            wq8, wk8, wv8, wo8 = w_packed[g]
            in_maps.append({
                "xt8": xt_packed[b],
                "wq8": wq8,
                "wk8": wk8,
                "wv8": wv8,
                "wo8": wo8,
                "alibi_b": ab,
                "alibi_q": alibi_q,
                "ones_col": ones_col,
                "tri": tri,
            })
    return in_maps


def kernel(x, Wq, Wk, Wv, Wo, _trace=False):
    from concourse.bass_utils import run_bass_kernel_spmd

    if "nc" not in _cache:
        _cache["nc"] = _build()
    nc = _cache["nc"]

    res = run_bass_kernel_spmd(
        nc, _in_maps(x, Wq, Wk, Wv, Wo), core_ids=list(range(2 * B)),
        trace=_trace)
    _cache["last_exec_time_ns"] = res.exec_time_ns

    out = np.empty((B, S, D), dtype=np.float32)
    for b in range(B):
        out[b] = (res.results[2 * b]["outT"] + res.results[2 * b + 1]["outT"]).T
    return out

